# revision 5
# baseline (speedup 1.0000x reference)
"""Multi-head attention (B=4, S=2048, D=1024, H=16) on 8 Trainium2 NeuronCores.

Sharding: tensor-parallel over heads. Core c owns heads 2c, 2c+1 (a 128-wide
slice of the model dim). Each core computes Q/K/V projections for its head
slice over all tokens, causal attention for its 2 heads, and a partial output
projection (contraction over its 128 x-dims). The host sums the 8 partial
outputs and adds b_o.

All matmuls run in fp16 (full PE rate) with fp32 PSUM accumulation; softmax
runs without max-subtraction (scores are O(10), exp stays in fp16 range).

On-device layouts (T = transposed, tokens on the free axis):
  QT/KT: [128 head-dims, 8192 tokens] fp16 in SBUF
  VA:    [128 token-chunk, 64 chunks, 256] fp16; cols 0-127 = V dims,
         cols 128-255 = ones (gives replicated softmax row-sums for free)
  Scores are computed transposed, S.T = [k-tokens, q-tokens], so softmax
  normalization lands on the free axis after the attn@V matmul.
"""

import sys
import types

sys.path.insert(0, "/opt/trn_rl_repo")

import numpy as np

# Optional: make run_bass_kernel_spmd(trace=True) work on images whose antenv
# lacks axon_hooks. Harmless if unavailable; kernel() defaults to trace=False.
try:  # pragma: no cover
    import antenv
    if "antenv.axon_hooks" not in sys.modules:
        from trn_agent_boot.trn_boot import _ntff_profile_via_ctypes

        _hook = _ntff_profile_via_ctypes("/opt/axon/libaxon_pjrt.so")
        _mod = types.ModuleType("antenv.axon_hooks")
        _mod.get_axon_ntff_profile_hook = lambda: _hook
        _mod.set_axon_ntff_profile_hook = lambda h: None
        sys.modules["antenv.axon_hooks"] = _mod
        antenv.axon_hooks = _mod
except Exception:
    pass

import concourse.bass as bass
import concourse.bacc as bacc
import concourse.tile as tile
import concourse.mybir as mybir
from concourse.bass_utils import run_bass_kernel_spmd

B, S, D, H = 4, 2048, 1024, 16
DK = D // H          # 64
P = 128
SQ = B * S           # 8192 tokens
NT = SQ // 512       # 16 token tiles of 512
KO = D // P          # 8 contraction chunks
NCORES = 8
F16 = mybir.dt.float16
F32 = mybir.dt.float32

TRACE = False        # set by test.py to capture an NTFF profile
LAST_RESULT = None   # BassKernelResults of the most recent run

_NC = None


def _build():
    nc = bacc.Bacc("TRN2", target_bir_lowering=False, debug=False,
                   num_devices=NCORES)

    qT_d = nc.dram_tensor("qT", [NT, P, KO, 512], F16, kind="ExternalInput")
    kT_d = nc.dram_tensor("kT", [NT, P, KO, 512], F16, kind="ExternalInput")
    vT_d = nc.dram_tensor("vT", [NT, P, KO, 512], F16, kind="ExternalInput")
    wq_d = nc.dram_tensor("wq", [P, KO, P], F16, kind="ExternalInput")
    wk_d = nc.dram_tensor("wk", [P, KO, P], F16, kind="ExternalInput")
    wv_d = nc.dram_tensor("wv", [P, KO, P], F16, kind="ExternalInput")
    wo_d = nc.dram_tensor("wo", [P, KO, P], F16, kind="ExternalInput")
    mk_d = nc.dram_tensor("masks", [P, 4, 512], F16, kind="ExternalInput")
    out_d = nc.dram_tensor("out", [KO, P, NT, 512], F16, kind="ExternalOutput")

    with tile.TileContext(nc) as tc:
        with (
            tc.tile_pool(name="const", bufs=1) as const,
            tc.tile_pool(name="persist", bufs=1) as persist,
            tc.tile_pool(name="stream", bufs=2) as stream,
            tc.tile_pool(name="epool", bufs=4) as epool,
            tc.tile_pool(name="rpool", bufs=4) as rpool,
            tc.tile_pool(name="ostage", bufs=4) as ostage,
            tc.tile_pool(name="pp", bufs=2, space="PSUM") as pp,
            tc.tile_pool(name="scp", bufs=4, space="PSUM") as scp,
            tc.tile_pool(name="opp", bufs=2, space="PSUM") as opp,
        ):
            wq_t = const.tile([P, KO, P], F16, tag="wq")
            wk_t = const.tile([P, KO, P], F16, tag="wk")
            wv_t = const.tile([P, KO, P], F16, tag="wv")
            wo_t = const.tile([P, KO, P], F16, tag="wo")
            mk_t = const.tile([P, 4, 512], F16, tag="mk")
            nc.sync.dma_start(wq_t[:], wq_d.ap())
            nc.sync.dma_start(wk_t[:], wk_d.ap())
            nc.sync.dma_start(wv_t[:], wv_d.ap())
            nc.sync.dma_start(wo_t[:], wo_d.ap())
            nc.sync.dma_start(mk_t[:], mk_d.ap())

            QT = persist.tile([P, SQ], F16, tag="QT")
            KT = persist.tile([P, SQ], F16, tag="KT")
            VA = persist.tile([P, SQ // P, 256], F16, tag="VA")
            XT = persist.tile([P, SQ], F16, tag="XT")

            # ones columns for the row-sum trick; per-chunk layout is
            # [A dims 0:64 | ones 64:128 | B dims 128:192 | ones 192:256]
            nc.vector.memset(VA[:, :, DK:P], 1.0)
            nc.vector.memset(VA[:, :, P + DK:2 * P], 1.0)

            def proj_tile(tt):
                """Q/K/V projections for token tile tt (512 tokens)."""
                cols = bass.ts(tt, 512)
                qin = stream.tile([P, KO, 512], F16, tag="qin")
                nc.sync.dma_start(qin[:], qT_d.ap()[tt])
                psq = pp.tile([P, 512], F32, tag="pp")
                for ko in range(KO):
                    nc.tensor.matmul(psq[:], wq_t[:, ko, :], qin[:, ko, :],
                                     start=(ko == 0), stop=(ko == KO - 1))
                nc.vector.tensor_copy(QT[:, cols], psq[:])

                kin = stream.tile([P, KO, 512], F16, tag="kin")
                nc.sync.dma_start(kin[:], kT_d.ap()[tt])
                psk = pp.tile([P, 512], F32, tag="pp")
                for ko in range(KO):
                    nc.tensor.matmul(psk[:], wk_t[:, ko, :], kin[:, ko, :],
                                     start=(ko == 0), stop=(ko == KO - 1))
                nc.vector.tensor_copy(KT[:, cols], psk[:])

                vin = stream.tile([P, KO, 512], F16, tag="vin")
                nc.sync.dma_start(vin[:], vT_d.ap()[tt])
                for sub in range(4):
                    psv = pp.tile([P, 512], F32, tag="pp")
                    for ko in range(KO):
                        nc.tensor.matmul(psv[:, 0:P],
                                         vin[:, ko, bass.ts(sub, P)],
                                         wv_t[:, ko, :],
                                         start=(ko == 0), stop=(ko == KO - 1))
                    kc = tt * 4 + sub
                    nc.vector.tensor_copy(VA[:, kc, 0:DK], psv[:, 0:DK])
                    nc.vector.tensor_copy(VA[:, kc, P:P + DK], psv[:, DK:P])

            def attention(b, qt):
                """One 512-query tile of causal attention, both heads."""
                qcols = bass.ds(b * S + qt * 512, 512)
                nkc = 4 * qt + 4
                ops = [opp.tile([P, 512], F32, tag="op", name=f"op{h}")
                       for h in range(2)]
                for kc in range(nkc):
                    gkc = b * (S // P) + kc
                    kcols = bass.ds(b * S + kc * P, P)
                    j = kc - 4 * qt
                    for h in range(2):
                        rb = h * DK
                        ssc = scp.tile([P, 512], F32, tag="sc")
                        nc.tensor.matmul(ssc[:], KT[rb:rb + DK, kcols],
                                         QT[rb:rb + DK, qcols],
                                         start=True, stop=True)
                        e_t = epool.tile([P, 512], F16, tag="e")
                        nc.scalar.activation(e_t[:], ssc[:],
                                             mybir.ActivationFunctionType.Exp,
                                             scale=0.125)
                        if j >= 0:
                            nc.vector.tensor_mul(e_t[:], e_t[:], mk_t[:, j, :])
                        nc.tensor.matmul(ops[h][:], VA[:, gkc, bass.ts(h, P)],
                                         e_t[:],
                                         start=(kc == 0), stop=(kc == nkc - 1))
                for h in range(2):
                    r_t = rpool.tile([DK, 512], F32, tag="r")
                    nc.vector.reciprocal(r_t[:], ops[h][DK:P, :])
                    nc.vector.tensor_mul(XT[h * DK:(h + 1) * DK, qcols],
                                         ops[h][0:DK, :], r_t[:])

            def oproj(b):
                """Partial output projection for batch b's 4 token tiles."""
                for mo in range(KO):
                    for tt in range(b * 4, b * 4 + 4):
                        pso = pp.tile([P, 512], F32, tag="pp")
                        nc.tensor.matmul(pso[:], wo_t[:, mo, :],
                                         XT[:, bass.ts(tt, 512)],
                                         start=True, stop=True)
                        ost = ostage.tile([P, 512], F16, tag="ost")
                        nc.vector.tensor_copy(ost[:], pso[:])
                        nc.sync.dma_start(out_d.ap()[mo, :, tt, :], ost[:])

            for b in range(B):
                for tt in range(b * 4, b * 4 + 4):
                    proj_tile(tt)
                for qt in range(4):
                    attention(b, qt)
                oproj(b)

    nc.compile()
    return nc


def _get_nc():
    global _NC
    if _NC is None:
        _NC = _build()
    return _NC


def _to_tiled_T(x2):
    """[SQ, D] fp32 -> [NT, 128, KO, 512] fp16 with x[g, d] at
    [g//512, d%128, d//128, g%512]."""
    xh = x2.astype(np.float16)
    return np.ascontiguousarray(
        xh.reshape(NT, 512, KO, P).transpose(0, 3, 2, 1))


def _weight_T(w_slice):
    """[128 out, 1024 in] -> [128 p, KO, 128 m] fp16 with W[m, d] at
    [d%128, d//128, m]."""
    return np.ascontiguousarray(
        w_slice.T.reshape(KO, P, P).transpose(1, 0, 2)).astype(np.float16)


def kernel(q, k, v, mask, W_q, W_k, W_v, W_o, b_o):
    global LAST_RESULT
    nc = _get_nc()

    qT = _to_tiled_T(np.asarray(q, np.float32).reshape(SQ, D))
    kT = _to_tiled_T(np.asarray(k, np.float32).reshape(SQ, D))
    vT = _to_tiled_T(np.asarray(v, np.float32).reshape(SQ, D))

    p_idx = np.arange(P)[:, None]
    f_idx = np.arange(512)[None, :]
    masks = np.stack(
        [(f_idx - j * P >= p_idx) for j in range(4)], 1).astype(np.float16)

    W_q = np.asarray(W_q, np.float32)
    W_k = np.asarray(W_k, np.float32)
    W_v = np.asarray(W_v, np.float32)
    W_o = np.asarray(W_o, np.float32)

    in_maps = []
    for c in range(NCORES):
        cs = slice(c * P, (c + 1) * P)
        in_maps.append({
            "qT": qT, "kT": kT, "vT": vT, "masks": masks,
            "wq": _weight_T(W_q[cs, :]),
            "wk": _weight_T(W_k[cs, :]),
            "wv": _weight_T(W_v[cs, :]),
            # [k, mo, m] = W_o[mo*128+m, c*128+k]
            "wo": np.ascontiguousarray(
                W_o[:, cs].reshape(KO, P, P).transpose(2, 0, 1)
            ).astype(np.float16),
        })

    res = run_bass_kernel_spmd(nc, in_maps, core_ids=list(range(NCORES)),
                               trace=TRACE)
    LAST_RESULT = res

    acc = np.zeros((SQ, D), np.float32)
    for c in range(NCORES):
        partial_T = res.results[c]["out"].reshape(D, SQ)
        acc += partial_T.T.astype(np.float32)
    acc += np.asarray(b_o, np.float32)
    return acc.reshape(B, S, D)


# revision 10
# speedup vs baseline: 1.0841x; 1.0841x over previous
"""Multi-head attention (B=4, S=2048, D=1024, H=16) on 8 Trainium2 NeuronCores.

Sharding: tensor-parallel over heads. Core c owns heads 2c, 2c+1 (a 128-wide
slice of the model dim). Each core computes Q/K/V projections for its head
slice over all tokens, causal attention for its 2 heads, and a partial output
projection (contraction over its 128 x-dims). The host sums the 8 partial
outputs and adds b_o.

All matmuls run in fp16 (full PE rate) with fp32 PSUM accumulation; softmax
runs without max-subtraction (scores are O(10), exp stays in fp16 range).

On-device layouts (T = transposed, tokens on the free axis):
  QT/KT: [128 head-dims, 8192 tokens] fp16 in SBUF
  VA:    [128 token-chunk, 64 chunks, 256] fp16; cols 0-127 = V dims,
         cols 128-255 = ones (gives replicated softmax row-sums for free)
  Scores are computed transposed, S.T = [k-tokens, q-tokens], so softmax
  normalization lands on the free axis after the attn@V matmul.
"""

import sys
import types

sys.path.insert(0, "/opt/trn_rl_repo")

import numpy as np

# Optional: make run_bass_kernel_spmd(trace=True) work on images whose antenv
# lacks axon_hooks. Harmless if unavailable; kernel() defaults to trace=False.
try:  # pragma: no cover
    import antenv
    if "antenv.axon_hooks" not in sys.modules:
        from trn_agent_boot.trn_boot import _ntff_profile_via_ctypes

        _hook = _ntff_profile_via_ctypes("/opt/axon/libaxon_pjrt.so")
        _mod = types.ModuleType("antenv.axon_hooks")
        _mod.get_axon_ntff_profile_hook = lambda: _hook
        _mod.set_axon_ntff_profile_hook = lambda h: None
        sys.modules["antenv.axon_hooks"] = _mod
        antenv.axon_hooks = _mod
except Exception:
    pass

import concourse.bass as bass
import concourse.bacc as bacc
import concourse.tile as tile
import concourse.mybir as mybir
from concourse.bass_utils import run_bass_kernel_spmd

B, S, D, H = 4, 2048, 1024, 16
DK = D // H          # 64
P = 128
SQ = B * S           # 8192 tokens
NT = SQ // 512       # 16 token tiles of 512
KO = D // P          # 8 contraction chunks
NCORES = 8
F16 = mybir.dt.float16
F32 = mybir.dt.float32

TRACE = False        # set by test.py to capture an NTFF profile
LAST_RESULT = None   # BassKernelResults of the most recent run

# Matmul operand dtype. bf16 runs the PE at full rate (fp16 is half-rate on
# TRN2); fp16 gives ~8x lower rounding error. Selected empirically.
MM_DT = mybir.dt.bfloat16
RECIP_FAST = False

_NC = None


def _np_mm_dt():
    if MM_DT == mybir.dt.float16:
        return np.float16
    import ml_dtypes
    return ml_dtypes.bfloat16


def _build():
    nc = bacc.Bacc("TRN2", target_bir_lowering=False, debug=False,
                   num_devices=NCORES)

    qT_d = nc.dram_tensor("qT", [NT, P, KO, 512], MM_DT, kind="ExternalInput")
    kT_d = nc.dram_tensor("kT", [NT, P, KO, 512], MM_DT, kind="ExternalInput")
    vT_d = nc.dram_tensor("vT", [NT, P, KO, 512], MM_DT, kind="ExternalInput")
    wq_d = nc.dram_tensor("wq", [P, KO, P], MM_DT, kind="ExternalInput")
    wk_d = nc.dram_tensor("wk", [P, KO, P], MM_DT, kind="ExternalInput")
    wv_d = nc.dram_tensor("wv", [P, KO, P], MM_DT, kind="ExternalInput")
    wo_d = nc.dram_tensor("wo", [P, KO, P], MM_DT, kind="ExternalInput")
    mk_d = nc.dram_tensor("masks", [P, 4, 512], MM_DT, kind="ExternalInput")
    out_d = nc.dram_tensor("out", [KO, P, NT, 512], F16, kind="ExternalOutput")

    with tile.TileContext(nc) as tc:
        with (
            tc.tile_pool(name="const", bufs=1) as const,
            tc.tile_pool(name="persist", bufs=1) as persist,
            tc.tile_pool(name="stream", bufs=2) as stream,
            tc.tile_pool(name="epool", bufs=4) as epool,
            tc.tile_pool(name="rpool", bufs=4) as rpool,
            tc.tile_pool(name="ostage", bufs=4) as ostage,
            tc.tile_pool(name="pp", bufs=2, space="PSUM") as pp,
            tc.tile_pool(name="scp", bufs=4, space="PSUM") as scp,
            tc.tile_pool(name="opp", bufs=2, space="PSUM") as opp,
        ):
            wq_t = const.tile([P, KO, P], MM_DT, tag="wq")
            wk_t = const.tile([P, KO, P], MM_DT, tag="wk")
            wv_t = const.tile([P, KO, P], MM_DT, tag="wv")
            wo_t = const.tile([P, KO, P], MM_DT, tag="wo")
            mk_t = const.tile([P, 4, 512], MM_DT, tag="mk")
            nc.sync.dma_start(wq_t[:], wq_d.ap())
            nc.sync.dma_start(wk_t[:], wk_d.ap())
            nc.sync.dma_start(wv_t[:], wv_d.ap())
            nc.sync.dma_start(wo_t[:], wo_d.ap())
            nc.sync.dma_start(mk_t[:], mk_d.ap())

            QT = persist.tile([P, SQ], MM_DT, tag="QT")
            KT = persist.tile([P, SQ], MM_DT, tag="KT")
            VA = persist.tile([P, SQ // P, 256], MM_DT, tag="VA")
            XT = persist.tile([P, SQ], MM_DT, tag="XT")

            # ones columns for the row-sum trick; per-chunk layout is
            # [A dims 0:64 | ones 64:128 | B dims 128:192 | ones 192:256]
            nc.vector.memset(VA[:, :, DK:P], 1.0)
            nc.vector.memset(VA[:, :, P + DK:2 * P], 1.0)

            def proj_tile(tt):
                """Q/K/V projections for token tile tt (512 tokens)."""
                cols = bass.ts(tt, 512)
                qin = stream.tile([P, KO, 512], MM_DT, tag="qin")
                nc.sync.dma_start(qin[:], qT_d.ap()[tt])
                psq = pp.tile([P, 512], F32, tag="pp")
                for ko in range(KO):
                    nc.tensor.matmul(psq[:], wq_t[:, ko, :], qin[:, ko, :],
                                     start=(ko == 0), stop=(ko == KO - 1))
                nc.vector.tensor_copy(QT[:, cols], psq[:])

                kin = stream.tile([P, KO, 512], MM_DT, tag="kin")
                nc.sync.dma_start(kin[:], kT_d.ap()[tt])
                psk = pp.tile([P, 512], F32, tag="pp")
                for ko in range(KO):
                    nc.tensor.matmul(psk[:], wk_t[:, ko, :], kin[:, ko, :],
                                     start=(ko == 0), stop=(ko == KO - 1))
                nc.vector.tensor_copy(KT[:, cols], psk[:])

                vin = stream.tile([P, KO, 512], MM_DT, tag="vin")
                nc.sync.dma_start(vin[:], vT_d.ap()[tt])
                for sub in range(4):
                    psv = pp.tile([P, 512], F32, tag="pp")
                    for ko in range(KO):
                        nc.tensor.matmul(psv[:, 0:P],
                                         vin[:, ko, bass.ts(sub, P)],
                                         wv_t[:, ko, :],
                                         start=(ko == 0), stop=(ko == KO - 1))
                    kc = tt * 4 + sub
                    nc.vector.tensor_copy(VA[:, kc, 0:DK], psv[:, 0:DK])
                    nc.vector.tensor_copy(VA[:, kc, P:P + DK], psv[:, DK:P])

            def attention(b, qt):
                """One 512-query tile of causal attention, both heads.

                Software-pipelined: the scores matmul + exp of chunk kc are
                emitted before the attn@V matmuls of chunk kc-1, so the PE
                never stalls on the ScalarE exp of the current chunk.
                """
                qcols = bass.ds(b * S + qt * 512, 512)
                nkc = 4 * qt + 4
                ops = [opp.tile([P, 512], F32, tag="op", name=f"op{h}")
                       for h in range(2)]
                e_prev = [None, None]

                def scores_exp(kc):
                    kcols = bass.ds(b * S + kc * P, P)
                    j = kc - 4 * qt
                    for h in range(2):
                        rb = h * DK
                        ssc = scp.tile([P, 512], F32, tag="sc", name="ssc")
                        nc.tensor.matmul(ssc[:], KT[rb:rb + DK, kcols],
                                         QT[rb:rb + DK, qcols],
                                         start=True, stop=True)
                        e_t = epool.tile([P, 512], MM_DT, tag="e", name="e_t")
                        nc.scalar.activation(e_t[:], ssc[:],
                                             mybir.ActivationFunctionType.Exp,
                                             scale=0.125)
                        if j >= 0:
                            nc.vector.tensor_mul(e_t[:], e_t[:], mk_t[:, j, :])
                        e_prev[h] = e_t

                def attn_mm(kc, e_pair):
                    gkc = b * (S // P) + kc
                    for h in range(2):
                        nc.tensor.matmul(ops[h][:], VA[:, gkc, bass.ts(h, P)],
                                         e_pair[h][:],
                                         start=(kc == 0), stop=(kc == nkc - 1))

                scores_exp(0)
                for kc in range(1, nkc):
                    pair = list(e_prev)
                    scores_exp(kc)
                    attn_mm(kc - 1, pair)
                attn_mm(nkc - 1, list(e_prev))

                for h in range(2):
                    r_t = rpool.tile([DK, 512], F32, tag="r", name="r_t")
                    if RECIP_FAST:
                        nc.vector.reciprocal_approx_fast(r_t[:],
                                                         ops[h][DK:P, :])
                    else:
                        nc.vector.reciprocal(r_t[:], ops[h][DK:P, :])
                    nc.vector.tensor_mul(XT[h * DK:(h + 1) * DK, qcols],
                                         ops[h][0:DK, :], r_t[:])

            def oproj(b):
                """Partial output projection for batch b's 4 token tiles."""
                for mo in range(KO):
                    for tt in range(b * 4, b * 4 + 4):
                        pso = pp.tile([P, 512], F32, tag="pp")
                        nc.tensor.matmul(pso[:], wo_t[:, mo, :],
                                         XT[:, bass.ts(tt, 512)],
                                         start=True, stop=True)
                        ost = ostage.tile([P, 512], F16, tag="ost")
                        nc.vector.tensor_copy(ost[:], pso[:])
                        nc.sync.dma_start(out_d.ap()[mo, :, tt, :], ost[:])

            for b in range(B):
                for tt in range(b * 4, b * 4 + 4):
                    proj_tile(tt)
                for qt in range(4):
                    attention(b, qt)
                oproj(b)

    nc.compile()
    return nc


def _get_nc():
    global _NC
    if _NC is None:
        _NC = _build()
    return _NC


def _to_tiled_T(x2):
    """[SQ, D] fp32 -> [NT, 128, KO, 512] fp16 with x[g, d] at
    [g//512, d%128, d//128, g%512]."""
    xh = x2.astype(_np_mm_dt())
    return np.ascontiguousarray(
        xh.reshape(NT, 512, KO, P).transpose(0, 3, 2, 1))


def _weight_T(w_slice):
    """[128 out, 1024 in] -> [128 p, KO, 128 m] fp16 with W[m, d] at
    [d%128, d//128, m]."""
    return np.ascontiguousarray(
        w_slice.T.reshape(KO, P, P).transpose(1, 0, 2)).astype(_np_mm_dt())


def kernel(q, k, v, mask, W_q, W_k, W_v, W_o, b_o):
    global LAST_RESULT
    nc = _get_nc()

    qT = _to_tiled_T(np.asarray(q, np.float32).reshape(SQ, D))
    kT = _to_tiled_T(np.asarray(k, np.float32).reshape(SQ, D))
    vT = _to_tiled_T(np.asarray(v, np.float32).reshape(SQ, D))

    p_idx = np.arange(P)[:, None]
    f_idx = np.arange(512)[None, :]
    masks = np.stack(
        [(f_idx - j * P >= p_idx) for j in range(4)], 1).astype(_np_mm_dt())

    W_q = np.asarray(W_q, np.float32)
    W_k = np.asarray(W_k, np.float32)
    W_v = np.asarray(W_v, np.float32)
    W_o = np.asarray(W_o, np.float32)

    in_maps = []
    for c in range(NCORES):
        cs = slice(c * P, (c + 1) * P)
        in_maps.append({
            "qT": qT, "kT": kT, "vT": vT, "masks": masks,
            "wq": _weight_T(W_q[cs, :]),
            "wk": _weight_T(W_k[cs, :]),
            "wv": _weight_T(W_v[cs, :]),
            # [k, mo, m] = W_o[mo*128+m, c*128+k]
            "wo": np.ascontiguousarray(
                W_o[:, cs].reshape(KO, P, P).transpose(2, 0, 1)
            ).astype(_np_mm_dt()),
        })

    res = run_bass_kernel_spmd(nc, in_maps, core_ids=list(range(NCORES)),
                               trace=TRACE)
    LAST_RESULT = res

    acc = np.zeros((SQ, D), np.float32)
    for c in range(NCORES):
        partial_T = res.results[c]["out"].reshape(D, SQ)
        acc += partial_T.T.astype(np.float32)
    acc += np.asarray(b_o, np.float32)
    return acc.reshape(B, S, D)


# revision 13
# speedup vs baseline: 1.2278x; 1.1325x over previous
"""Multi-head attention (B=4, S=2048, D=1024, H=16) on 8 Trainium2 NeuronCores.

Sharding: tensor-parallel over heads. Core c owns heads 2c, 2c+1 (a 128-wide
slice of the model dim). Each core computes Q/K/V projections for its head
slice over all tokens, causal attention for its 2 heads, and a partial output
projection (contraction over its 128 x-dims). The host sums the 8 partial
outputs and adds b_o.

All matmuls run in fp16 (full PE rate) with fp32 PSUM accumulation; softmax
runs without max-subtraction (scores are O(10), exp stays in fp16 range).

On-device layouts (T = transposed, tokens on the free axis):
  QT/KT: [128 head-dims, 8192 tokens] fp16 in SBUF
  VA:    [128 token-chunk, 64 chunks, 256] fp16; cols 0-127 = V dims,
         cols 128-255 = ones (gives replicated softmax row-sums for free)
  Scores are computed transposed, S.T = [k-tokens, q-tokens], so softmax
  normalization lands on the free axis after the attn@V matmul.
"""

import sys
import types

sys.path.insert(0, "/opt/trn_rl_repo")

import numpy as np

# Optional: make run_bass_kernel_spmd(trace=True) work on images whose antenv
# lacks axon_hooks. Harmless if unavailable; kernel() defaults to trace=False.
try:  # pragma: no cover
    import antenv
    if "antenv.axon_hooks" not in sys.modules:
        from trn_agent_boot.trn_boot import _ntff_profile_via_ctypes

        _hook = _ntff_profile_via_ctypes("/opt/axon/libaxon_pjrt.so")
        _mod = types.ModuleType("antenv.axon_hooks")
        _mod.get_axon_ntff_profile_hook = lambda: _hook
        _mod.set_axon_ntff_profile_hook = lambda h: None
        sys.modules["antenv.axon_hooks"] = _mod
        antenv.axon_hooks = _mod
except Exception:
    pass

import concourse.bass as bass
import concourse.bacc as bacc
import concourse.tile as tile
import concourse.mybir as mybir
from concourse.bass_utils import run_bass_kernel_spmd

B, S, D, H = 4, 2048, 1024, 16
DK = D // H          # 64
P = 128
SQ = B * S           # 8192 tokens
NT = SQ // 512       # 16 token tiles of 512
KO = D // P          # 8 contraction chunks
NCORES = 8
F16 = mybir.dt.float16
F32 = mybir.dt.float32

TRACE = False        # set by test.py to capture an NTFF profile
LAST_RESULT = None   # BassKernelResults of the most recent run

# Matmul operand dtype. bf16 runs the PE at full rate (fp16 is half-rate on
# TRN2); fp16 gives ~8x lower rounding error. Selected empirically.
MM_DT = mybir.dt.bfloat16
RECIP_FAST = True

_NC = None


def _np_mm_dt():
    if MM_DT == mybir.dt.float16:
        return np.float16
    import ml_dtypes
    return ml_dtypes.bfloat16


def _build():
    nc = bacc.Bacc("TRN2", target_bir_lowering=False, debug=False,
                   num_devices=NCORES)

    qT_d = nc.dram_tensor("qT", [NT, P, KO, 512], MM_DT, kind="ExternalInput")
    kT_d = nc.dram_tensor("kT", [NT, P, KO, 512], MM_DT, kind="ExternalInput")
    vT_d = nc.dram_tensor("vT", [NT, P, KO, 512], MM_DT, kind="ExternalInput")
    wq_d = nc.dram_tensor("wq", [P, KO, P], MM_DT, kind="ExternalInput")
    wk_d = nc.dram_tensor("wk", [P, KO, P], MM_DT, kind="ExternalInput")
    wv_d = nc.dram_tensor("wv", [P, KO, P], MM_DT, kind="ExternalInput")
    wo_d = nc.dram_tensor("wo", [P, KO, P], MM_DT, kind="ExternalInput")
    mk_d = nc.dram_tensor("masks", [P, P], MM_DT, kind="ExternalInput")
    out_d = nc.dram_tensor("out", [KO, P, NT, 512], F16, kind="ExternalOutput")

    with tile.TileContext(nc) as tc:
        with (
            tc.tile_pool(name="const", bufs=1) as const,
            tc.tile_pool(name="persist", bufs=1) as persist,
            tc.tile_pool(name="stream", bufs=2) as stream,
            tc.tile_pool(name="epool", bufs=4) as epool,
            tc.tile_pool(name="rpool", bufs=4) as rpool,
            tc.tile_pool(name="ostage", bufs=4) as ostage,
            tc.tile_pool(name="vstage", bufs=2) as vstage,
            tc.tile_pool(name="pp", bufs=2, space="PSUM") as pp,
            tc.tile_pool(name="scp", bufs=4, space="PSUM") as scp,
            tc.tile_pool(name="opp", bufs=2, space="PSUM") as opp,
        ):
            wq_t = const.tile([P, KO, P], MM_DT, tag="wq")
            wk_t = const.tile([P, KO, P], MM_DT, tag="wk")
            wv_t = const.tile([P, KO, P], MM_DT, tag="wv")
            wo_t = const.tile([P, KO, P], MM_DT, tag="wo")
            mk_t = const.tile([P, P], MM_DT, tag="mk")
            nc.sync.dma_start(wq_t[:], wq_d.ap())
            nc.sync.dma_start(wk_t[:], wk_d.ap())
            nc.sync.dma_start(wv_t[:], wv_d.ap())
            nc.sync.dma_start(wo_t[:], wo_d.ap())
            nc.sync.dma_start(mk_t[:], mk_d.ap())

            ident = const.tile([P, P], MM_DT, tag="ident")
            from concourse.masks import make_identity
            make_identity(nc, ident[:])

            QT = persist.tile([P, SQ], MM_DT, tag="QT")
            KT = persist.tile([P, SQ], MM_DT, tag="KT")
            VA = persist.tile([P, SQ // P, 256], MM_DT, tag="VA")
            XT = persist.tile([P, SQ], MM_DT, tag="XT")

            # ones columns for the row-sum trick; per-chunk layout is
            # [A dims 0:64 | ones 64:128 | B dims 128:192 | ones 192:256]
            nc.vector.memset(VA[:, :, DK:P], 1.0)
            nc.vector.memset(VA[:, :, P + DK:2 * P], 1.0)

            def proj_tile(tt):
                """Q/K/V projections for token tile tt (512 tokens)."""
                cols = bass.ts(tt, 512)
                qin = stream.tile([P, KO, 512], MM_DT, tag="qin")
                nc.sync.dma_start(qin[:], qT_d.ap()[tt])
                psq = pp.tile([P, 512], F32, tag="pp")
                for ko in range(KO):
                    nc.tensor.matmul(psq[:], wq_t[:, ko, :], qin[:, ko, :],
                                     start=(ko == 0), stop=(ko == KO - 1))
                nc.vector.tensor_copy(QT[:, cols], psq[:])

                kin = stream.tile([P, KO, 512], MM_DT, tag="kin")
                nc.sync.dma_start(kin[:], kT_d.ap()[tt])
                psk = pp.tile([P, 512], F32, tag="pp")
                for ko in range(KO):
                    nc.tensor.matmul(psk[:], wk_t[:, ko, :], kin[:, ko, :],
                                     start=(ko == 0), stop=(ko == KO - 1))
                nc.vector.tensor_copy(KT[:, cols], psk[:])

                vin = stream.tile([P, KO, 512], MM_DT, tag="vin")
                nc.sync.dma_start(vin[:], vT_d.ap()[tt])
                psv = pp.tile([P, 512], F32, tag="pp")
                for ko in range(KO):
                    nc.tensor.matmul(psv[:], wv_t[:, ko, :], vin[:, ko, :],
                                     start=(ko == 0), stop=(ko == KO - 1))
                vts = vstage.tile([P, 512], MM_DT, tag="vts")
                nc.vector.tensor_copy(vts[:], psv[:])
                for sub in range(4):
                    tp = pp.tile([P, P], MM_DT, tag="pp", name="tp")
                    nc.tensor.transpose(tp[:], vts[:, bass.ts(sub, P)],
                                        ident[:])
                    kc = tt * 4 + sub
                    nc.vector.tensor_copy(VA[:, kc, 0:DK], tp[:, 0:DK])
                    nc.vector.tensor_copy(VA[:, kc, P:P + DK], tp[:, DK:P])

            def attention(b, qt):
                """One 512-query tile of causal attention, both heads.

                Software-pipelined: the scores matmul + exp of chunk kc are
                emitted before the attn@V matmuls of chunk kc-1, so the PE
                never stalls on the ScalarE exp of the current chunk.
                """
                qcols = bass.ds(b * S + qt * 512, 512)
                nkc = 4 * qt + 4
                ops = [opp.tile([P, 512], F32, tag="op", name=f"op{h}")
                       for h in range(2)]
                e_prev = [None, None]

                def scores_exp(kc):
                    kcols = bass.ds(b * S + kc * P, P)
                    j = kc - 4 * qt
                    co = max(j, 0) * P  # valid q-columns start here (causal)
                    w = 512 - co
                    for h in range(2):
                        rb = h * DK
                        ssc = scp.tile([P, 512], F32, tag="sc", name="ssc")
                        nc.tensor.matmul(
                            ssc[:, co:],
                            KT[rb:rb + DK, kcols],
                            QT[rb:rb + DK, bass.ds(b * S + qt * 512 + co, w)],
                            start=True, stop=True)
                        e_t = epool.tile([P, 512], MM_DT, tag="e", name="e_t")
                        nc.scalar.activation(e_t[:, co:], ssc[:, co:],
                                             mybir.ActivationFunctionType.Exp,
                                             scale=0.125)
                        if j >= 0:
                            nc.vector.tensor_mul(e_t[:, co:co + P],
                                                 e_t[:, co:co + P], mk_t[:])
                        e_prev[h] = (e_t, co)

                def attn_mm(kc, e_pair):
                    gkc = b * (S // P) + kc
                    for h in range(2):
                        e_t, co = e_pair[h]
                        nc.tensor.matmul(ops[h][:, co:],
                                         VA[:, gkc, bass.ts(h, P)],
                                         e_t[:, co:],
                                         start=(kc == 0), stop=(kc == nkc - 1))

                scores_exp(0)
                for kc in range(1, nkc):
                    pair = list(e_prev)
                    scores_exp(kc)
                    attn_mm(kc - 1, pair)
                attn_mm(nkc - 1, list(e_prev))

                for h in range(2):
                    r_t = rpool.tile([DK, 512], F32, tag="r", name="r_t")
                    if RECIP_FAST:
                        s_t = rpool.tile([DK, 512], F32, tag="s", name="s_t")
                        nc.vector.tensor_copy(s_t[:], ops[h][DK:P, :])
                        nc.vector.reciprocal_approx_fast(r_t[:], s_t[:])
                    else:
                        nc.vector.reciprocal(r_t[:], ops[h][DK:P, :])
                    nc.vector.tensor_mul(XT[h * DK:(h + 1) * DK, qcols],
                                         ops[h][0:DK, :], r_t[:])

            def oproj(b):
                """Partial output projection for batch b's 4 token tiles."""
                for mo in range(KO):
                    for tt in range(b * 4, b * 4 + 4):
                        pso = pp.tile([P, 512], F32, tag="pp")
                        nc.tensor.matmul(pso[:], wo_t[:, mo, :],
                                         XT[:, bass.ts(tt, 512)],
                                         start=True, stop=True)
                        ost = ostage.tile([P, 512], F16, tag="ost")
                        nc.vector.tensor_copy(ost[:], pso[:])
                        nc.sync.dma_start(out_d.ap()[mo, :, tt, :], ost[:])

            for b in range(B):
                for tt in range(b * 4, b * 4 + 4):
                    proj_tile(tt)
                for qt in range(4):
                    attention(b, qt)
                oproj(b)

    nc.compile()
    return nc


def _get_nc():
    global _NC
    if _NC is None:
        _NC = _build()
    return _NC


def _to_tiled_T(x2):
    """[SQ, D] fp32 -> [NT, 128, KO, 512] fp16 with x[g, d] at
    [g//512, d%128, d//128, g%512]."""
    xh = x2.astype(_np_mm_dt())
    return np.ascontiguousarray(
        xh.reshape(NT, 512, KO, P).transpose(0, 3, 2, 1))


def _weight_T(w_slice):
    """[128 out, 1024 in] -> [128 p, KO, 128 m] fp16 with W[m, d] at
    [d%128, d//128, m]."""
    return np.ascontiguousarray(
        w_slice.T.reshape(KO, P, P).transpose(1, 0, 2)).astype(_np_mm_dt())


def kernel(q, k, v, mask, W_q, W_k, W_v, W_o, b_o):
    global LAST_RESULT
    nc = _get_nc()

    qT = _to_tiled_T(np.asarray(q, np.float32).reshape(SQ, D))
    kT = _to_tiled_T(np.asarray(k, np.float32).reshape(SQ, D))
    vT = _to_tiled_T(np.asarray(v, np.float32).reshape(SQ, D))

    p_idx = np.arange(P)[:, None]
    f_idx = np.arange(P)[None, :]
    masks = (f_idx >= p_idx).astype(_np_mm_dt())

    W_q = np.asarray(W_q, np.float32)
    W_k = np.asarray(W_k, np.float32)
    W_v = np.asarray(W_v, np.float32)
    W_o = np.asarray(W_o, np.float32)

    in_maps = []
    for c in range(NCORES):
        cs = slice(c * P, (c + 1) * P)
        in_maps.append({
            "qT": qT, "kT": kT, "vT": vT, "masks": masks,
            "wq": _weight_T(W_q[cs, :]),
            "wk": _weight_T(W_k[cs, :]),
            "wv": _weight_T(W_v[cs, :]),
            # [k, mo, m] = W_o[mo*128+m, c*128+k]
            "wo": np.ascontiguousarray(
                W_o[:, cs].reshape(KO, P, P).transpose(2, 0, 1)
            ).astype(_np_mm_dt()),
        })

    res = run_bass_kernel_spmd(nc, in_maps, core_ids=list(range(NCORES)),
                               trace=TRACE)
    LAST_RESULT = res

    acc = np.zeros((SQ, D), np.float32)
    for c in range(NCORES):
        partial_T = res.results[c]["out"].reshape(D, SQ)
        acc += partial_T.T.astype(np.float32)
    acc += np.asarray(b_o, np.float32)
    return acc.reshape(B, S, D)


# revision 15
# speedup vs baseline: 1.5251x; 1.2421x over previous
"""Multi-head attention (B=4, S=2048, D=1024, H=16) on 8 Trainium2 NeuronCores.

Sharding: tensor-parallel over heads. Core c owns heads 2c, 2c+1 (a 128-wide
slice of the model dim). Each core computes Q/K/V projections for its head
slice over all tokens, causal attention for its 2 heads, and a partial output
projection (contraction over its 128 x-dims). The host sums the 8 partial
outputs and adds b_o.

All matmuls run in fp16 (full PE rate) with fp32 PSUM accumulation; softmax
runs without max-subtraction (scores are O(10), exp stays in fp16 range).

On-device layouts (T = transposed, tokens on the free axis):
  QT/KT: [128 head-dims, 8192 tokens] fp16 in SBUF
  VA:    [128 token-chunk, 64 chunks, 256] fp16; cols 0-127 = V dims,
         cols 128-255 = ones (gives replicated softmax row-sums for free)
  Scores are computed transposed, S.T = [k-tokens, q-tokens], so softmax
  normalization lands on the free axis after the attn@V matmul.
"""

import sys
import types

sys.path.insert(0, "/opt/trn_rl_repo")

import numpy as np

# Optional: make run_bass_kernel_spmd(trace=True) work on images whose antenv
# lacks axon_hooks. Harmless if unavailable; kernel() defaults to trace=False.
try:  # pragma: no cover
    import antenv
    if "antenv.axon_hooks" not in sys.modules:
        from trn_agent_boot.trn_boot import _ntff_profile_via_ctypes

        _hook = _ntff_profile_via_ctypes("/opt/axon/libaxon_pjrt.so")
        _mod = types.ModuleType("antenv.axon_hooks")
        _mod.get_axon_ntff_profile_hook = lambda: _hook
        _mod.set_axon_ntff_profile_hook = lambda h: None
        sys.modules["antenv.axon_hooks"] = _mod
        antenv.axon_hooks = _mod
except Exception:
    pass

import concourse.bass as bass
import concourse.bacc as bacc
import concourse.tile as tile
import concourse.mybir as mybir
from concourse.bass_utils import run_bass_kernel_spmd

B, S, D, H = 4, 2048, 1024, 16
DK = D // H          # 64
P = 128
SQ = B * S           # 8192 tokens
NT = SQ // 512       # 16 token tiles of 512
KO = D // P          # 8 contraction chunks
NCORES = 8
F16 = mybir.dt.float16
F32 = mybir.dt.float32

TRACE = False        # set by test.py to capture an NTFF profile
LAST_RESULT = None   # BassKernelResults of the most recent run

# Matmul operand dtype. bf16 runs the PE at full rate (fp16 is half-rate on
# TRN2); fp16 gives ~8x lower rounding error. Selected empirically.
MM_DT = mybir.dt.bfloat16
RECIP_FAST = True

_NC = None


def _np_mm_dt():
    if MM_DT == mybir.dt.float16:
        return np.float16
    import ml_dtypes
    return ml_dtypes.bfloat16


def _build():
    nc = bacc.Bacc("TRN2", target_bir_lowering=False, debug=False,
                   num_devices=NCORES)

    qT_d = nc.dram_tensor("qT", [NT, P, KO, 512], MM_DT, kind="ExternalInput")
    kT_d = nc.dram_tensor("kT", [NT, P, KO, 512], MM_DT, kind="ExternalInput")
    vT_d = nc.dram_tensor("vT", [NT, P, KO, 512], MM_DT, kind="ExternalInput")
    wq_d = nc.dram_tensor("wq", [P, KO, P], MM_DT, kind="ExternalInput")
    wk_d = nc.dram_tensor("wk", [P, KO, P], MM_DT, kind="ExternalInput")
    wv_d = nc.dram_tensor("wv", [P, KO, P], MM_DT, kind="ExternalInput")
    wo_d = nc.dram_tensor("wo", [P, KO, P], MM_DT, kind="ExternalInput")
    mk_d = nc.dram_tensor("masks", [P, P], MM_DT, kind="ExternalInput")
    out_d = nc.dram_tensor("out", [KO, P, NT, 512], F16, kind="ExternalOutput")

    with tile.TileContext(nc) as tc:
        with (
            tc.tile_pool(name="const", bufs=1) as const,
            tc.tile_pool(name="persist", bufs=1) as persist,
            tc.tile_pool(name="stream", bufs=3) as stream,
            tc.tile_pool(name="epool", bufs=6) as epool,
            tc.tile_pool(name="rpool", bufs=4) as rpool,
            tc.tile_pool(name="ostage", bufs=4) as ostage,
            tc.tile_pool(name="vstage", bufs=2) as vstage,
            tc.tile_pool(name="pp", bufs=2, space="PSUM") as pp,
            tc.tile_pool(name="scp", bufs=4, space="PSUM") as scp,
            tc.tile_pool(name="opp", bufs=2, space="PSUM") as opp,
        ):
            wq_t = const.tile([P, KO, P], MM_DT, tag="wq")
            wk_t = const.tile([P, KO, P], MM_DT, tag="wk")
            wv_t = const.tile([P, KO, P], MM_DT, tag="wv")
            wo_t = const.tile([P, KO, P], MM_DT, tag="wo")
            mk_t = const.tile([P, P], MM_DT, tag="mk")
            nc.sync.dma_start(wq_t[:], wq_d.ap())
            nc.sync.dma_start(wk_t[:], wk_d.ap())
            nc.sync.dma_start(wv_t[:], wv_d.ap())
            nc.sync.dma_start(wo_t[:], wo_d.ap())
            nc.sync.dma_start(mk_t[:], mk_d.ap())

            ident = const.tile([P, P], MM_DT, tag="ident")
            from concourse.masks import make_identity
            make_identity(nc, ident[:])

            QT = persist.tile([P, SQ], MM_DT, tag="QT")
            KT = persist.tile([P, SQ], MM_DT, tag="KT")
            VA = persist.tile([P, SQ // P, 256], MM_DT, tag="VA")
            XT = persist.tile([P, SQ], MM_DT, tag="XT")

            # ones columns for the row-sum trick; per-chunk layout is
            # [A dims 0:64 | ones 64:128 | B dims 128:192 | ones 192:256]
            nc.vector.memset(VA[:, :, DK:P], 1.0)
            nc.vector.memset(VA[:, :, P + DK:2 * P], 1.0)

            def proj_tile(tt):
                """Q/K/V projections for token tile tt (512 tokens)."""
                cols = bass.ts(tt, 512)
                qin = stream.tile([P, KO, 512], MM_DT, tag="qin")
                nc.sync.dma_start(qin[:], qT_d.ap()[tt])
                psq = pp.tile([P, 512], F32, tag="pp")
                for ko in range(KO):
                    nc.tensor.matmul(psq[:], wq_t[:, ko, :], qin[:, ko, :],
                                     start=(ko == 0), stop=(ko == KO - 1))
                nc.vector.tensor_copy(QT[:, cols], psq[:])

                kin = stream.tile([P, KO, 512], MM_DT, tag="kin")
                nc.sync.dma_start(kin[:], kT_d.ap()[tt])
                psk = pp.tile([P, 512], F32, tag="pp")
                for ko in range(KO):
                    nc.tensor.matmul(psk[:], wk_t[:, ko, :], kin[:, ko, :],
                                     start=(ko == 0), stop=(ko == KO - 1))
                nc.vector.tensor_copy(KT[:, cols], psk[:])

                vin = stream.tile([P, KO, 512], MM_DT, tag="vin")
                nc.sync.dma_start(vin[:], vT_d.ap()[tt])
                psv = pp.tile([P, 512], F32, tag="pp")
                for ko in range(KO):
                    nc.tensor.matmul(psv[:], wv_t[:, ko, :], vin[:, ko, :],
                                     start=(ko == 0), stop=(ko == KO - 1))
                vts = vstage.tile([P, 512], MM_DT, tag="vts")
                nc.vector.tensor_copy(vts[:], psv[:])
                for sub in range(4):
                    tp = pp.tile([P, P], MM_DT, tag="pp", name="tp")
                    nc.tensor.transpose(tp[:], vts[:, bass.ts(sub, P)],
                                        ident[:])
                    kc = tt * 4 + sub
                    nc.vector.tensor_copy(VA[:, kc, 0:DK], tp[:, 0:DK])
                    nc.vector.tensor_copy(VA[:, kc, P:P + DK], tp[:, DK:P])

            def attention(b, qt):
                """One 512-query tile of causal attention, both heads.

                Software-pipelined: the scores matmul + exp of chunk kc are
                emitted before the attn@V matmuls of chunk kc-1, so the PE
                never stalls on the ScalarE exp of the current chunk.
                """
                qcols = bass.ds(b * S + qt * 512, 512)
                nkc = 4 * qt + 4
                ops = [opp.tile([P, 512], F32, tag="op", name=f"op{h}")
                       for h in range(2)]
                e_prev = [None, None]

                def scores_exp(kc):
                    kcols = bass.ds(b * S + kc * P, P)
                    j = kc - 4 * qt
                    co = max(j, 0) * P  # valid q-columns start here (causal)
                    w = 512 - co
                    for h in range(2):
                        rb = h * DK
                        ssc = scp.tile([P, 512], F32, tag="sc", name="ssc")
                        nc.tensor.matmul(
                            ssc[:, co:],
                            KT[rb:rb + DK, kcols],
                            QT[rb:rb + DK, bass.ds(b * S + qt * 512 + co, w)],
                            start=True, stop=True)
                        e_t = epool.tile([P, 512], MM_DT, tag="e", name="e_t")
                        nc.scalar.activation(e_t[:, co:], ssc[:, co:],
                                             mybir.ActivationFunctionType.Exp,
                                             scale=0.125)
                        if j >= 0:
                            nc.vector.tensor_mul(e_t[:, co:co + P],
                                                 e_t[:, co:co + P], mk_t[:])
                        e_prev[h] = (e_t, co)

                def attn_mm(kc, e_pair):
                    gkc = b * (S // P) + kc
                    for h in range(2):
                        e_t, co = e_pair[h]
                        nc.tensor.matmul(ops[h][:, co:],
                                         VA[:, gkc, bass.ts(h, P)],
                                         e_t[:, co:],
                                         start=(kc == 0), stop=(kc == nkc - 1))

                scores_exp(0)
                for kc in range(1, nkc):
                    pair = list(e_prev)
                    scores_exp(kc)
                    attn_mm(kc - 1, pair)
                attn_mm(nkc - 1, list(e_prev))

                for h in range(2):
                    r_t = rpool.tile([DK, 512], F32, tag="r", name="r_t")
                    if RECIP_FAST:
                        s_t = rpool.tile([DK, 512], F32, tag="s", name="s_t")
                        nc.vector.tensor_copy(s_t[:], ops[h][DK:P, :])
                        nc.vector.reciprocal_approx_fast(r_t[:], s_t[:])
                    else:
                        nc.vector.reciprocal(r_t[:], ops[h][DK:P, :])
                    nc.vector.tensor_mul(XT[h * DK:(h + 1) * DK, qcols],
                                         ops[h][0:DK, :], r_t[:])

            def oproj(b):
                """Partial output projection for batch b's 4 token tiles."""
                for mo in range(KO):
                    for tt in range(b * 4, b * 4 + 4):
                        pso = pp.tile([P, 512], F32, tag="pp")
                        nc.tensor.matmul(pso[:], wo_t[:, mo, :],
                                         XT[:, bass.ts(tt, 512)],
                                         start=True, stop=True)
                        ost = ostage.tile([P, 512], F16, tag="ost")
                        nc.vector.tensor_copy(ost[:], pso[:])
                        nc.sync.dma_start(out_d.ap()[mo, :, tt, :], ost[:])

            # Batch-0 projections up front; later batches' projections are
            # interleaved into the previous batch's attention so the PE
            # stream stays dense (keeps the HAM clock-gate at 2.4 GHz) and
            # exp-wait bubbles are filled with independent matmul work.
            for tt in range(4):
                proj_tile(tt)
            for b in range(B):
                for qt in range(4):
                    attention(b, qt)
                    if b + 1 < B:
                        proj_tile(4 * (b + 1) + qt)
                oproj(b)

    nc.compile()
    return nc


def _get_nc():
    global _NC
    if _NC is None:
        _NC = _build()
    return _NC


def _to_tiled_T(x2):
    """[SQ, D] fp32 -> [NT, 128, KO, 512] fp16 with x[g, d] at
    [g//512, d%128, d//128, g%512]."""
    xh = x2.astype(_np_mm_dt())
    return np.ascontiguousarray(
        xh.reshape(NT, 512, KO, P).transpose(0, 3, 2, 1))


def _weight_T(w_slice):
    """[128 out, 1024 in] -> [128 p, KO, 128 m] fp16 with W[m, d] at
    [d%128, d//128, m]."""
    return np.ascontiguousarray(
        w_slice.T.reshape(KO, P, P).transpose(1, 0, 2)).astype(_np_mm_dt())


def kernel(q, k, v, mask, W_q, W_k, W_v, W_o, b_o):
    global LAST_RESULT
    nc = _get_nc()

    qT = _to_tiled_T(np.asarray(q, np.float32).reshape(SQ, D))
    kT = _to_tiled_T(np.asarray(k, np.float32).reshape(SQ, D))
    vT = _to_tiled_T(np.asarray(v, np.float32).reshape(SQ, D))

    p_idx = np.arange(P)[:, None]
    f_idx = np.arange(P)[None, :]
    masks = (f_idx >= p_idx).astype(_np_mm_dt())

    W_q = np.asarray(W_q, np.float32)
    W_k = np.asarray(W_k, np.float32)
    W_v = np.asarray(W_v, np.float32)
    W_o = np.asarray(W_o, np.float32)

    in_maps = []
    for c in range(NCORES):
        cs = slice(c * P, (c + 1) * P)
        in_maps.append({
            "qT": qT, "kT": kT, "vT": vT, "masks": masks,
            "wq": _weight_T(W_q[cs, :]),
            "wk": _weight_T(W_k[cs, :]),
            "wv": _weight_T(W_v[cs, :]),
            # [k, mo, m] = W_o[mo*128+m, c*128+k]
            "wo": np.ascontiguousarray(
                W_o[:, cs].reshape(KO, P, P).transpose(2, 0, 1)
            ).astype(_np_mm_dt()),
        })

    res = run_bass_kernel_spmd(nc, in_maps, core_ids=list(range(NCORES)),
                               trace=TRACE)
    LAST_RESULT = res

    acc = np.zeros((SQ, D), np.float32)
    for c in range(NCORES):
        partial_T = res.results[c]["out"].reshape(D, SQ)
        acc += partial_T.T.astype(np.float32)
    acc += np.asarray(b_o, np.float32)
    return acc.reshape(B, S, D)


# revision 17
# speedup vs baseline: 1.6805x; 1.1019x over previous
"""Multi-head attention (B=4, S=2048, D=1024, H=16) on 8 Trainium2 NeuronCores.

Sharding: tensor-parallel over heads. Core c owns heads 2c, 2c+1 (a 128-wide
slice of the model dim). Each core computes Q/K/V projections for its head
slice over all tokens, causal attention for its 2 heads, and a partial output
projection (contraction over its 128 x-dims). The host sums the 8 partial
outputs and adds b_o.

All matmuls run in fp16 (full PE rate) with fp32 PSUM accumulation; softmax
runs without max-subtraction (scores are O(10), exp stays in fp16 range).

On-device layouts (T = transposed, tokens on the free axis):
  QT/KT: [128 head-dims, 8192 tokens] fp16 in SBUF
  VA:    [128 token-chunk, 64 chunks, 256] fp16; cols 0-127 = V dims,
         cols 128-255 = ones (gives replicated softmax row-sums for free)
  Scores are computed transposed, S.T = [k-tokens, q-tokens], so softmax
  normalization lands on the free axis after the attn@V matmul.
"""

import sys
import types

sys.path.insert(0, "/opt/trn_rl_repo")

import numpy as np

# Optional: make run_bass_kernel_spmd(trace=True) work on images whose antenv
# lacks axon_hooks. Harmless if unavailable; kernel() defaults to trace=False.
try:  # pragma: no cover
    import antenv
    if "antenv.axon_hooks" not in sys.modules:
        from trn_agent_boot.trn_boot import _ntff_profile_via_ctypes

        _hook = _ntff_profile_via_ctypes("/opt/axon/libaxon_pjrt.so")
        _mod = types.ModuleType("antenv.axon_hooks")
        _mod.get_axon_ntff_profile_hook = lambda: _hook
        _mod.set_axon_ntff_profile_hook = lambda h: None
        sys.modules["antenv.axon_hooks"] = _mod
        antenv.axon_hooks = _mod
except Exception:
    pass

import concourse.bass as bass
import concourse.bacc as bacc
import concourse.tile as tile
import concourse.mybir as mybir
from concourse.bass_utils import run_bass_kernel_spmd

B, S, D, H = 4, 2048, 1024, 16
DK = D // H          # 64
P = 128
SQ = B * S           # 8192 tokens
NT = SQ // 512       # 16 token tiles of 512
KO = D // P          # 8 contraction chunks
NCORES = 8
F16 = mybir.dt.float16
F32 = mybir.dt.float32

TRACE = False        # set by test.py to capture an NTFF profile
LAST_RESULT = None   # BassKernelResults of the most recent run

# Matmul operand dtype. bf16 runs the PE at full rate (fp16 is half-rate on
# TRN2); fp16 gives ~8x lower rounding error. Selected empirically.
MM_DT = mybir.dt.bfloat16
RECIP_FAST = True

_NC = None


def _np_mm_dt():
    if MM_DT == mybir.dt.float16:
        return np.float16
    import ml_dtypes
    return ml_dtypes.bfloat16


def _build():
    nc = bacc.Bacc("TRN2", target_bir_lowering=False, debug=False,
                   num_devices=NCORES)

    qT_d = nc.dram_tensor("qT", [NT, P, KO, 512], MM_DT, kind="ExternalInput")
    kT_d = nc.dram_tensor("kT", [NT, P, KO, 512], MM_DT, kind="ExternalInput")
    vT_d = nc.dram_tensor("vT", [NT, P, KO, 512], MM_DT, kind="ExternalInput")
    wq_d = nc.dram_tensor("wq", [P, KO, P], MM_DT, kind="ExternalInput")
    wk_d = nc.dram_tensor("wk", [P, KO, P], MM_DT, kind="ExternalInput")
    wv_d = nc.dram_tensor("wv", [P, KO, P], MM_DT, kind="ExternalInput")
    wo_d = nc.dram_tensor("wo", [P, KO, P], MM_DT, kind="ExternalInput")
    mk_d = nc.dram_tensor("masks", [P, P], MM_DT, kind="ExternalInput")
    out_d = nc.dram_tensor("out", [KO, P, NT, 512], F16, kind="ExternalOutput")

    with tile.TileContext(nc) as tc:
        with (
            tc.tile_pool(name="const", bufs=1) as const,
            tc.tile_pool(name="persist", bufs=1) as persist,
            tc.tile_pool(name="stream", bufs=3) as stream,
            tc.tile_pool(name="epool", bufs=6) as epool,
            tc.tile_pool(name="rpool", bufs=4) as rpool,
            tc.tile_pool(name="ostage", bufs=4) as ostage,
            tc.tile_pool(name="vstage", bufs=2) as vstage,
            tc.tile_pool(name="pp", bufs=2, space="PSUM") as pp,
            tc.tile_pool(name="scp", bufs=4, space="PSUM") as scp,
            tc.tile_pool(name="opp", bufs=2, space="PSUM") as opp,
        ):
            wq_t = const.tile([P, KO, P], MM_DT, tag="wq")
            wk_t = const.tile([P, KO, P], MM_DT, tag="wk")
            wv_t = const.tile([P, KO, P], MM_DT, tag="wv")
            wo_t = const.tile([P, KO, P], MM_DT, tag="wo")
            mk_t = const.tile([P, P], MM_DT, tag="mk")
            nc.sync.dma_start(wq_t[:], wq_d.ap())
            nc.sync.dma_start(wk_t[:], wk_d.ap())
            nc.sync.dma_start(wv_t[:], wv_d.ap())
            nc.sync.dma_start(wo_t[:], wo_d.ap())
            nc.sync.dma_start(mk_t[:], mk_d.ap())

            ident = const.tile([P, P], MM_DT, tag="ident")
            from concourse.masks import make_identity
            make_identity(nc, ident[:])

            QT = persist.tile([P, SQ], MM_DT, tag="QT")
            KT = persist.tile([P, SQ], MM_DT, tag="KT")
            VA = persist.tile([P, SQ // P, 256], MM_DT, tag="VA")
            XT = persist.tile([P, SQ], MM_DT, tag="XT")

            # ones columns for the row-sum trick; per-chunk layout is
            # [A dims 0:64 | ones 64:128 | B dims 128:192 | ones 192:256]
            nc.vector.memset(VA[:, :, DK:P], 1.0)
            nc.vector.memset(VA[:, :, P + DK:2 * P], 1.0)

            def proj_tile(tt):
                """Q/K/V projections for token tile tt (512 tokens)."""
                cols = bass.ts(tt, 512)
                qin = stream.tile([P, KO, 512], MM_DT, tag="qin")
                nc.sync.dma_start(qin[:], qT_d.ap()[tt])
                psq = pp.tile([P, 512], F32, tag="pp")
                for ko in range(KO):
                    nc.tensor.matmul(psq[:], wq_t[:, ko, :], qin[:, ko, :],
                                     start=(ko == 0), stop=(ko == KO - 1))
                nc.vector.tensor_copy(QT[:, cols], psq[:])

                kin = stream.tile([P, KO, 512], MM_DT, tag="kin")
                nc.sync.dma_start(kin[:], kT_d.ap()[tt])
                psk = pp.tile([P, 512], F32, tag="pp")
                for ko in range(KO):
                    nc.tensor.matmul(psk[:], wk_t[:, ko, :], kin[:, ko, :],
                                     start=(ko == 0), stop=(ko == KO - 1))
                nc.vector.tensor_copy(KT[:, cols], psk[:])

                vin = stream.tile([P, KO, 512], MM_DT, tag="vin")
                nc.sync.dma_start(vin[:], vT_d.ap()[tt])
                psv = pp.tile([P, 512], F32, tag="pp")
                for ko in range(KO):
                    nc.tensor.matmul(psv[:], wv_t[:, ko, :], vin[:, ko, :],
                                     start=(ko == 0), stop=(ko == KO - 1))
                vts = vstage.tile([P, 512], MM_DT, tag="vts")
                nc.vector.tensor_copy(vts[:], psv[:])
                for sub in range(4):
                    tp = pp.tile([P, P], MM_DT, tag="pp", name="tp")
                    nc.tensor.transpose(tp[:], vts[:, bass.ts(sub, P)],
                                        ident[:])
                    kc = tt * 4 + sub
                    nc.vector.tensor_copy(VA[:, kc, 0:DK], tp[:, 0:DK])
                    nc.vector.tensor_copy(VA[:, kc, P:P + DK], tp[:, DK:P])

            def attention(b, qt):
                """One 512-query tile of causal attention, both heads.

                Software-pipelined: the scores matmul + exp of chunk kc are
                emitted before the attn@V matmuls of chunk kc-1, so the PE
                never stalls on the ScalarE exp of the current chunk.
                """
                qcols = bass.ds(b * S + qt * 512, 512)
                nkc = 4 * qt + 4
                ops = [opp.tile([P, 512], F32, tag="op", name=f"op{h}")
                       for h in range(2)]
                e_prev = [None, None]

                def scores_exp(kc):
                    kcols = bass.ds(b * S + kc * P, P)
                    j = kc - 4 * qt
                    co = max(j, 0) * P  # valid q-columns start here (causal)
                    w = 512 - co
                    for h in range(2):
                        rb = h * DK
                        ssc = scp.tile([P, 512], F32, tag="sc", name="ssc")
                        nc.tensor.matmul(
                            ssc[:, co:],
                            KT[rb:rb + DK, kcols],
                            QT[rb:rb + DK, bass.ds(b * S + qt * 512 + co, w)],
                            start=True, stop=True)
                        e_t = epool.tile([P, 512], MM_DT, tag="e", name="e_t")
                        nc.scalar.activation(e_t[:, co:], ssc[:, co:],
                                             mybir.ActivationFunctionType.Exp,
                                             scale=0.125)
                        if j >= 0:
                            nc.vector.tensor_mul(e_t[:, co:co + P],
                                                 e_t[:, co:co + P], mk_t[:])
                        e_prev[h] = (e_t, co)

                def attn_mm(kc, e_pair):
                    gkc = b * (S // P) + kc
                    for h in range(2):
                        e_t, co = e_pair[h]
                        nc.tensor.matmul(ops[h][:, co:],
                                         VA[:, gkc, bass.ts(h, P)],
                                         e_t[:, co:],
                                         start=(kc == 0), stop=(kc == nkc - 1))

                scores_exp(0)
                for kc in range(1, nkc):
                    pair = list(e_prev)
                    scores_exp(kc)
                    attn_mm(kc - 1, pair)
                attn_mm(nkc - 1, list(e_prev))

                for h in range(2):
                    r_t = rpool.tile([DK, 512], F32, tag="r", name="r_t")
                    if RECIP_FAST:
                        s_t = rpool.tile([DK, 512], F32, tag="s", name="s_t")
                        nc.vector.tensor_copy(s_t[:], ops[h][DK:P, :])
                        nc.vector.reciprocal_approx_fast(r_t[:], s_t[:])
                    else:
                        nc.vector.reciprocal(r_t[:], ops[h][DK:P, :])
                    nc.vector.tensor_mul(XT[h * DK:(h + 1) * DK, qcols],
                                         ops[h][0:DK, :], r_t[:])

            def oproj_quarter(b, i):
                """2 of 8 output-dim chunks of batch b's output projection."""
                for mo in range(2 * i, 2 * i + 2):
                    for tt in range(b * 4, b * 4 + 4):
                        pso = pp.tile([P, 512], F32, tag="pp")
                        nc.tensor.matmul(pso[:], wo_t[:, mo, :],
                                         XT[:, bass.ts(tt, 512)],
                                         start=True, stop=True)
                        ost = ostage.tile([P, 512], F16, tag="ost")
                        nc.vector.tensor_copy(ost[:], pso[:])
                        nc.sync.dma_start(out_d.ap()[mo, :, tt, :], ost[:])

            # Batch-0 projections up front; later batches' projections and
            # the previous batch's output projection are interleaved into the
            # attention stream so the PE stays dense (keeps the HAM
            # clock-gate at 2.4 GHz) and exp-wait bubbles are filled with
            # independent matmul work.
            for tt in range(4):
                proj_tile(tt)
            for b in range(B):
                for qt in range(4):
                    attention(b, qt)
                    if b + 1 < B:
                        proj_tile(4 * (b + 1) + qt)
                    if b > 0:
                        oproj_quarter(b - 1, qt)
            for i in range(4):
                oproj_quarter(B - 1, i)

    nc.compile()
    return nc


def _get_nc():
    global _NC
    if _NC is None:
        _NC = _build()
    return _NC


def _to_tiled_T(x2):
    """[SQ, D] fp32 -> [NT, 128, KO, 512] fp16 with x[g, d] at
    [g//512, d%128, d//128, g%512]."""
    xh = x2.astype(_np_mm_dt())
    return np.ascontiguousarray(
        xh.reshape(NT, 512, KO, P).transpose(0, 3, 2, 1))


def _weight_T(w_slice):
    """[128 out, 1024 in] -> [128 p, KO, 128 m] fp16 with W[m, d] at
    [d%128, d//128, m]."""
    return np.ascontiguousarray(
        w_slice.T.reshape(KO, P, P).transpose(1, 0, 2)).astype(_np_mm_dt())


def kernel(q, k, v, mask, W_q, W_k, W_v, W_o, b_o):
    global LAST_RESULT
    nc = _get_nc()

    qT = _to_tiled_T(np.asarray(q, np.float32).reshape(SQ, D))
    kT = _to_tiled_T(np.asarray(k, np.float32).reshape(SQ, D))
    vT = _to_tiled_T(np.asarray(v, np.float32).reshape(SQ, D))

    p_idx = np.arange(P)[:, None]
    f_idx = np.arange(P)[None, :]
    masks = (f_idx >= p_idx).astype(_np_mm_dt())

    W_q = np.asarray(W_q, np.float32)
    W_k = np.asarray(W_k, np.float32)
    W_v = np.asarray(W_v, np.float32)
    W_o = np.asarray(W_o, np.float32)

    in_maps = []
    for c in range(NCORES):
        cs = slice(c * P, (c + 1) * P)
        in_maps.append({
            "qT": qT, "kT": kT, "vT": vT, "masks": masks,
            "wq": _weight_T(W_q[cs, :]),
            "wk": _weight_T(W_k[cs, :]),
            "wv": _weight_T(W_v[cs, :]),
            # [k, mo, m] = W_o[mo*128+m, c*128+k]
            "wo": np.ascontiguousarray(
                W_o[:, cs].reshape(KO, P, P).transpose(2, 0, 1)
            ).astype(_np_mm_dt()),
        })

    res = run_bass_kernel_spmd(nc, in_maps, core_ids=list(range(NCORES)),
                               trace=TRACE)
    LAST_RESULT = res

    acc = np.zeros((SQ, D), np.float32)
    for c in range(NCORES):
        partial_T = res.results[c]["out"].reshape(D, SQ)
        acc += partial_T.T.astype(np.float32)
    acc += np.asarray(b_o, np.float32)
    return acc.reshape(B, S, D)


# revision 19
# speedup vs baseline: 1.7356x; 1.0328x over previous
"""Multi-head attention (B=4, S=2048, D=1024, H=16) on 8 Trainium2 NeuronCores.

Sharding: tensor-parallel over heads. Core c owns heads 2c, 2c+1 (a 128-wide
slice of the model dim). Each core computes Q/K/V projections for its head
slice over all tokens, causal attention for its 2 heads, and a partial output
projection (contraction over its 128 x-dims). The host sums the 8 partial
outputs and adds b_o.

All matmuls run in fp16 (full PE rate) with fp32 PSUM accumulation; softmax
runs without max-subtraction (scores are O(10), exp stays in fp16 range).

On-device layouts (T = transposed, tokens on the free axis):
  QT/KT: [128 head-dims, 8192 tokens] fp16 in SBUF
  VA:    [128 token-chunk, 64 chunks, 256] fp16; cols 0-127 = V dims,
         cols 128-255 = ones (gives replicated softmax row-sums for free)
  Scores are computed transposed, S.T = [k-tokens, q-tokens], so softmax
  normalization lands on the free axis after the attn@V matmul.
"""

import sys
import types

sys.path.insert(0, "/opt/trn_rl_repo")

import numpy as np

# Optional: make run_bass_kernel_spmd(trace=True) work on images whose antenv
# lacks axon_hooks. Harmless if unavailable; kernel() defaults to trace=False.
try:  # pragma: no cover
    import antenv
    if "antenv.axon_hooks" not in sys.modules:
        from trn_agent_boot.trn_boot import _ntff_profile_via_ctypes

        _hook = _ntff_profile_via_ctypes("/opt/axon/libaxon_pjrt.so")
        _mod = types.ModuleType("antenv.axon_hooks")
        _mod.get_axon_ntff_profile_hook = lambda: _hook
        _mod.set_axon_ntff_profile_hook = lambda h: None
        sys.modules["antenv.axon_hooks"] = _mod
        antenv.axon_hooks = _mod
except Exception:
    pass

import concourse.bass as bass
import concourse.bacc as bacc
import concourse.tile as tile
import concourse.mybir as mybir
from concourse.bass_utils import run_bass_kernel_spmd

B, S, D, H = 4, 2048, 1024, 16
DK = D // H          # 64
P = 128
SQ = B * S           # 8192 tokens
NT = SQ // 512       # 16 token tiles of 512
KO = D // P          # 8 contraction chunks
NCORES = 8
F16 = mybir.dt.float16
F32 = mybir.dt.float32

TRACE = False        # set by test.py to capture an NTFF profile
LAST_RESULT = None   # BassKernelResults of the most recent run

# Matmul operand dtype. bf16 runs the PE at full rate (fp16 is half-rate on
# TRN2); fp16 gives ~8x lower rounding error. Selected empirically.
MM_DT = mybir.dt.bfloat16
RECIP_FAST = True

_NC = None


def _np_mm_dt():
    if MM_DT == mybir.dt.float16:
        return np.float16
    import ml_dtypes
    return ml_dtypes.bfloat16


def _build():
    nc = bacc.Bacc("TRN2", target_bir_lowering=False, debug=False,
                   num_devices=NCORES)

    qT_d = nc.dram_tensor("qT", [NT, P, KO, 512], MM_DT, kind="ExternalInput")
    kT_d = nc.dram_tensor("kT", [NT, P, KO, 512], MM_DT, kind="ExternalInput")
    vT_d = nc.dram_tensor("vT", [NT, P, KO, 512], MM_DT, kind="ExternalInput")
    wq_d = nc.dram_tensor("wq", [P, KO, P], MM_DT, kind="ExternalInput")
    wk_d = nc.dram_tensor("wk", [P, KO, P], MM_DT, kind="ExternalInput")
    wv_d = nc.dram_tensor("wv", [P, KO, P], MM_DT, kind="ExternalInput")
    wo_d = nc.dram_tensor("wo", [P, KO, P], MM_DT, kind="ExternalInput")
    mk_d = nc.dram_tensor("masks", [P, P], MM_DT, kind="ExternalInput")
    out_d = nc.dram_tensor("out", [KO, P, NT, 512], F16, kind="ExternalOutput")

    with tile.TileContext(nc) as tc:
        with (
            tc.tile_pool(name="const", bufs=1) as const,
            tc.tile_pool(name="persist", bufs=1) as persist,
            tc.tile_pool(name="stream", bufs=3) as stream,
            tc.tile_pool(name="epool", bufs=6) as epool,
            tc.tile_pool(name="rpool", bufs=4) as rpool,
            tc.tile_pool(name="ostage", bufs=4) as ostage,
            tc.tile_pool(name="vstage", bufs=2) as vstage,
            tc.tile_pool(name="pp", bufs=2, space="PSUM") as pp,
            tc.tile_pool(name="scp", bufs=4, space="PSUM") as scp,
            tc.tile_pool(name="opp", bufs=2, space="PSUM") as opp,
        ):
            wq_t = const.tile([P, KO, P], MM_DT, tag="wq")
            wk_t = const.tile([P, KO, P], MM_DT, tag="wk")
            wv_t = const.tile([P, KO, P], MM_DT, tag="wv")
            wo_t = const.tile([P, KO, P], MM_DT, tag="wo")
            mk_t = const.tile([P, P], MM_DT, tag="mk")
            nc.sync.dma_start(wq_t[:], wq_d.ap())
            nc.sync.dma_start(wk_t[:], wk_d.ap())
            nc.sync.dma_start(wv_t[:], wv_d.ap())
            nc.sync.dma_start(wo_t[:], wo_d.ap())
            nc.sync.dma_start(mk_t[:], mk_d.ap())

            ident = const.tile([P, P], MM_DT, tag="ident")
            from concourse.masks import make_identity
            make_identity(nc, ident[:])

            QT = persist.tile([P, SQ], MM_DT, tag="QT")
            KT = persist.tile([P, SQ], MM_DT, tag="KT")
            VA = persist.tile([P, SQ // P, 256], MM_DT, tag="VA")
            XT = persist.tile([P, SQ], MM_DT, tag="XT")

            # ones columns for the row-sum trick; per-chunk layout is
            # [A dims 0:64 | ones 64:128 | B dims 128:192 | ones 192:256]
            nc.vector.memset(VA[:, :, DK:P], 1.0)
            nc.vector.memset(VA[:, :, P + DK:2 * P], 1.0)

            def proj_tile(tt):
                """Q/K/V projections for token tile tt (512 tokens)."""
                cols = bass.ts(tt, 512)
                qin = stream.tile([P, KO, 512], MM_DT, tag="qin")
                nc.sync.dma_start(qin[:], qT_d.ap()[tt])
                psq = pp.tile([P, 512], F32, tag="pp")
                for ko in range(KO):
                    nc.tensor.matmul(psq[:], wq_t[:, ko, :], qin[:, ko, :],
                                     start=(ko == 0), stop=(ko == KO - 1))
                nc.vector.tensor_copy(QT[:, cols], psq[:])

                kin = stream.tile([P, KO, 512], MM_DT, tag="kin")
                nc.sync.dma_start(kin[:], kT_d.ap()[tt])
                psk = pp.tile([P, 512], F32, tag="pp")
                for ko in range(KO):
                    nc.tensor.matmul(psk[:], wk_t[:, ko, :], kin[:, ko, :],
                                     start=(ko == 0), stop=(ko == KO - 1))
                nc.vector.tensor_copy(KT[:, cols], psk[:])

                vin = stream.tile([P, KO, 512], MM_DT, tag="vin")
                nc.sync.dma_start(vin[:], vT_d.ap()[tt])
                psv = pp.tile([P, 512], F32, tag="pp")
                for ko in range(KO):
                    nc.tensor.matmul(psv[:], wv_t[:, ko, :], vin[:, ko, :],
                                     start=(ko == 0), stop=(ko == KO - 1))
                vts = vstage.tile([P, 512], MM_DT, tag="vts")
                nc.vector.tensor_copy(vts[:], psv[:])
                for sub in range(4):
                    tp = pp.tile([P, P], MM_DT, tag="pp", name="tp")
                    nc.tensor.transpose(tp[:], vts[:, bass.ts(sub, P)],
                                        ident[:])
                    kc = tt * 4 + sub
                    # one strided copy: head halves land at cols 0:64, 128:192
                    nc.vector.tensor_copy(
                        VA[:, kc].rearrange("p (a x) -> p a x", a=2)[:, :, 0:DK],
                        tp[:].rearrange("p (a x) -> p a x", a=2))

            def attention(b, qt):
                """One 512-query tile of causal attention, both heads.

                Software-pipelined: the scores matmul + exp of chunk kc are
                emitted before the attn@V matmuls of chunk kc-1, so the PE
                never stalls on the ScalarE exp of the current chunk.
                """
                qcols = bass.ds(b * S + qt * 512, 512)
                nkc = 4 * qt + 4
                ops = [opp.tile([P, 512], F32, tag="op", name=f"op{h}")
                       for h in range(2)]
                e_prev = [None, None]

                def scores_exp(kc):
                    kcols = bass.ds(b * S + kc * P, P)
                    j = kc - 4 * qt
                    co = max(j, 0) * P  # valid q-columns start here (causal)
                    w = 512 - co
                    for h in range(2):
                        rb = h * DK
                        ssc = scp.tile([P, 512], F32, tag="sc", name="ssc")
                        nc.tensor.matmul(
                            ssc[:, co:],
                            KT[rb:rb + DK, kcols],
                            QT[rb:rb + DK, bass.ds(b * S + qt * 512 + co, w)],
                            start=True, stop=True)
                        e_t = epool.tile([P, 512], MM_DT, tag="e", name="e_t")
                        nc.scalar.activation(e_t[:, co:], ssc[:, co:],
                                             mybir.ActivationFunctionType.Exp,
                                             scale=0.125)
                        if j >= 0:
                            nc.gpsimd.tensor_mul(e_t[:, co:co + P],
                                                 e_t[:, co:co + P], mk_t[:])
                        e_prev[h] = (e_t, co)

                def attn_mm(kc, e_pair):
                    gkc = b * (S // P) + kc
                    for h in range(2):
                        e_t, co = e_pair[h]
                        nc.tensor.matmul(ops[h][:, co:],
                                         VA[:, gkc, bass.ts(h, P)],
                                         e_t[:, co:],
                                         start=(kc == 0), stop=(kc == nkc - 1))

                scores_exp(0)
                for kc in range(1, nkc):
                    pair = list(e_prev)
                    scores_exp(kc)
                    attn_mm(kc - 1, pair)
                attn_mm(nkc - 1, list(e_prev))

                for h in range(2):
                    r_t = rpool.tile([DK, 512], F32, tag="r", name="r_t")
                    if RECIP_FAST:
                        s_t = rpool.tile([DK, 512], F32, tag="s", name="s_t")
                        nc.vector.tensor_copy(s_t[:], ops[h][DK:P, :])
                        nc.vector.reciprocal_approx_fast(r_t[:], s_t[:])
                    else:
                        nc.vector.reciprocal(r_t[:], ops[h][DK:P, :])
                    nc.vector.tensor_mul(XT[h * DK:(h + 1) * DK, qcols],
                                         ops[h][0:DK, :], r_t[:])

            def oproj_quarter(b, i):
                """2 of 8 output-dim chunks of batch b's output projection."""
                for mo in range(2 * i, 2 * i + 2):
                    for tt in range(b * 4, b * 4 + 4):
                        pso = pp.tile([P, 512], F32, tag="pp")
                        nc.tensor.matmul(pso[:], wo_t[:, mo, :],
                                         XT[:, bass.ts(tt, 512)],
                                         start=True, stop=True)
                        ost = ostage.tile([P, 512], F16, tag="ost")
                        nc.vector.tensor_copy(ost[:], pso[:])
                        nc.sync.dma_start(out_d.ap()[mo, :, tt, :], ost[:])

            # Batch-0 projections up front; later batches' projections and
            # the previous batch's output projection are interleaved into the
            # attention stream so the PE stays dense (keeps the HAM
            # clock-gate at 2.4 GHz) and exp-wait bubbles are filled with
            # independent matmul work.
            for tt in range(4):
                proj_tile(tt)
            for b in range(B):
                for qt in range(4):
                    attention(b, qt)
                    if b + 1 < B:
                        proj_tile(4 * (b + 1) + qt)
                    if b > 0:
                        oproj_quarter(b - 1, qt)
            for i in range(4):
                oproj_quarter(B - 1, i)

    nc.compile()
    return nc


def _get_nc():
    global _NC
    if _NC is None:
        _NC = _build()
    return _NC


def _to_tiled_T(x2):
    """[SQ, D] fp32 -> [NT, 128, KO, 512] fp16 with x[g, d] at
    [g//512, d%128, d//128, g%512]."""
    xh = x2.astype(_np_mm_dt())
    return np.ascontiguousarray(
        xh.reshape(NT, 512, KO, P).transpose(0, 3, 2, 1))


def _weight_T(w_slice):
    """[128 out, 1024 in] -> [128 p, KO, 128 m] fp16 with W[m, d] at
    [d%128, d//128, m]."""
    return np.ascontiguousarray(
        w_slice.T.reshape(KO, P, P).transpose(1, 0, 2)).astype(_np_mm_dt())


def kernel(q, k, v, mask, W_q, W_k, W_v, W_o, b_o):
    global LAST_RESULT
    nc = _get_nc()

    qT = _to_tiled_T(np.asarray(q, np.float32).reshape(SQ, D))
    kT = _to_tiled_T(np.asarray(k, np.float32).reshape(SQ, D))
    vT = _to_tiled_T(np.asarray(v, np.float32).reshape(SQ, D))

    p_idx = np.arange(P)[:, None]
    f_idx = np.arange(P)[None, :]
    masks = (f_idx >= p_idx).astype(_np_mm_dt())

    W_q = np.asarray(W_q, np.float32)
    W_k = np.asarray(W_k, np.float32)
    W_v = np.asarray(W_v, np.float32)
    W_o = np.asarray(W_o, np.float32)

    in_maps = []
    for c in range(NCORES):
        cs = slice(c * P, (c + 1) * P)
        in_maps.append({
            "qT": qT, "kT": kT, "vT": vT, "masks": masks,
            "wq": _weight_T(W_q[cs, :]),
            "wk": _weight_T(W_k[cs, :]),
            "wv": _weight_T(W_v[cs, :]),
            # [k, mo, m] = W_o[mo*128+m, c*128+k]
            "wo": np.ascontiguousarray(
                W_o[:, cs].reshape(KO, P, P).transpose(2, 0, 1)
            ).astype(_np_mm_dt()),
        })

    res = run_bass_kernel_spmd(nc, in_maps, core_ids=list(range(NCORES)),
                               trace=TRACE)
    LAST_RESULT = res

    acc = np.zeros((SQ, D), np.float32)
    for c in range(NCORES):
        partial_T = res.results[c]["out"].reshape(D, SQ)
        acc += partial_T.T.astype(np.float32)
    acc += np.asarray(b_o, np.float32)
    return acc.reshape(B, S, D)
